# revision 1
# baseline (speedup 1.0000x reference)
"""Trainium2 Bass kernel for a KAN layer (512->512, cubic B-spline, 17 ctrl pts).

Math: out[b,o] = sum_i w_b[i,o]*silu(xt[i,b]) + sum_i sum_c D[i,o,c]*B3_c(v[i,b])
with xt = clip(x.T, -bound, bound), v = (xt-g0)/h, D = w_s[:,:,None]*control_points.

The cubic B-spline basis over a uniform grid is rewritten via the truncated-power
identity  N3(s) = (1/6) * sum_m (-1)^m C(4,m) relu(s-m)^3, so the whole layer
collapses into ONE GEMM over K = 1 + 9*512 rows:
  [silu | u | u^2 | u^3 | relu(t-k3)^3 .. relu(t-k7)^3 | ones]   (u = t centered)
against host-folded weights [w_b | G1 | G2 | G3 | E3..E7 | Gsum0].
Relu^3 pieces with knots below the clip range never truncate and fold into the
centered global cubic (G*); pieces with knots above it vanish.

Sharding: data-parallel over batch, 512 rows per core x 8 cores. The GEMM is
computed as out^T = features^T @ W (features stationary, weights moving, fp32
data issued as float32r so the PE runs at 1 cycle/row), so the output lands
b-major and stores contiguously.

TRN2 TPB instructions carry a single sync-wait slot, so the dataflow is built
so every instruction depends on at most one foreign semaphore: each K-block's
weight tile is staged through a copy on the block's feature-producing engine,
weight tiles are not pool-recycled (no PE release waits), and output stores go
through SWDGE.
"""

import os
import sys
from math import comb

import numpy as np

for _p in ("/opt/trn_rl_repo",):
    if os.path.isdir(_p) and _p not in sys.path:
        sys.path.insert(0, _p)

BATCH, IN_DIM, OUT_DIM, NCORES = 4096, 512, 512, 8
BC = BATCH // NCORES  # 512 batch rows per core
NKT = 37  # K tiles: 9 blocks * 4 tiles + 1 padded "ones" tile
NUM_CTRL = 17

# "f32r": fp32 data, matmuls issued as float32r (1 cyc/row). "f32": exact fp32.
MM_DTYPE = os.environ.get("KAN_MM_DTYPE", "bf16")

_nc_cache: dict = {}


def _build_nc(g0: float, h: float, bound: float):
    import concourse.bass as bass
    import concourse.mybir as mybir
    import concourse.tile as tile

    f32 = mybir.dt.float32
    f32r = mybir.dt.float32r
    AF = mybir.ActivationFunctionType
    ALU = mybir.AluOpType

    bf16 = mybir.dt.bfloat16
    fmm = {"f32r": f32r, "bf16": bf16, "f32": f32}[MM_DTYPE]
    tctr = g0 + 5.0 * h  # data-range center in t-units (0.0 for the default grid)
    knots = [g0 + k * h for k in range(3, 8)]

    nc = bass.Bass()
    xt_d = nc.dram_tensor("xt", [4, 128, BC], f32, kind="ExternalInput")
    w_d = nc.dram_tensor("w", [NKT + 1, 128, OUT_DIM], fmm, kind="ExternalInput")
    out_d = nc.dram_tensor("out", [4, 128, OUT_DIM], f32, kind="ExternalOutput")

    with tile.TileContext(nc) as tc:
        with (
            tc.tile_pool(name="data", bufs=1) as datap,
            tc.tile_pool(name="wt", bufs=1) as wp,
            tc.tile_pool(name="psum", bufs=1, space="PSUM") as pp,
        ):
            xt = datap.tile([128, 4, BC], f32, name="xt_sb")
            nc.sync.dma_start(xt[:], xt_d[:].rearrange("g p b -> p g b"))

            # All weights arrive via ONE striped cast-DMA on the SWDGE queue;
            # the ones-feature rides the same queue, so the first matmul of
            # the K loop needs exactly one sync wait (that queue's sem).
            wbig = wp.tile([128, NKT + 1, OUT_DIM], fmm, name="wbig")
            nc.sync.dma_start(wbig[:], w_d[:].rearrange("k p o -> p k o"))


            _consts = {}

            def cbias(val: float):
                if val == 0.0:
                    return 0.0
                if val not in _consts:
                    ct = datap.tile([128, 1], f32, name=f"c{len(_consts)}")
                    nc.vector.memset(ct[:], val)
                    _consts[val] = ct
                return _consts[val][:]

            tc_t = datap.tile([128, 4, BC], f32, name="tc")
            nc.vector.tensor_scalar(tc_t[:], xt[:], -bound, bound, ALU.max, ALU.min)

            # ACT-produced feature blocks (0..2); paired weight copies on ACT.
            silu_t = datap.tile([128, 4, BC], fmm, name="silu")
            nc.scalar.activation(silu_t[:], tc_t[:], AF.Silu)
            u_t = datap.tile([128, 4, BC], fmm, name="u")
            nc.scalar.activation(u_t[:], tc_t[:], AF.Copy, bias=-tctr)
            u2_t = datap.tile([128, 4, BC], fmm, name="u2")
            nc.scalar.activation(u2_t[:], tc_t[:], AF.Square, bias=cbias(-tctr))
            # DVE-produced blocks (3..8); paired weight copies on DVE.
            u3_t = datap.tile([128, 4, BC], fmm, name="u3")
            nc.vector.tensor_mul(u3_t[:], u2_t[:], u_t[:])

            feat_tiles = [silu_t, u_t, u2_t, u3_t]
            for j, kn in enumerate(knots):
                r = datap.tile([128, 4, BC], fmm, name=f"r{j}")
                nc.scalar.activation(r[:], tc_t[:], AF.Relu, bias=cbias(-kn))
                r2 = datap.tile([128, 4, BC], fmm, name=f"r2_{j}")
                nc.vector.tensor_mul(r2[:], r[:], r[:])
                r3 = datap.tile([128, 4, BC], fmm, name=f"r3_{j}")
                nc.vector.tensor_mul(r3[:], r2[:], r[:])
                feat_tiles.append(r3)

            psums = [pp.tile([128, OUT_DIM], f32, name=f"ps{m}") for m in range(4)]
            for kt2 in range(NKT):
                # ones block first: its matmuls wait only on the SWDGE queue
                # sem (which also covers wbig); later matmuls wait only on
                # their feature tile's engine sem.
                kt = (kt2 + NKT - 1) % NKT
                for m in range(4):
                    if kt == NKT - 1:
                        lhsT = wbig[:, NKT, m * 128 : (m + 1) * 128]
                    else:
                        blk, gi = kt // 4, kt % 4
                        lhsT = feat_tiles[blk][:, gi, m * 128 : (m + 1) * 128]
                    nc.tensor.matmul(
                        psums[m][:],
                        lhsT,
                        wbig[:, kt, :],
                        start=(kt2 == 0),
                        stop=(kt2 == NKT - 1),
                    )

            osb = datap.tile([128, 4, OUT_DIM], f32, name="osb")
            for m in range(4):
                nc.scalar.copy(osb[:, m, :], psums[m][:])
            nc.sync.dma_start(out_d[:].rearrange("g p o -> p g o"), osb[:])

    # The Tile kernel-tail drain waits on every proc's sem (6 waits), but the
    # TPB Drain encoding holds fewer. All dataflow here funnels into the single
    # output-store DMA: its completion transitively implies PE/ACT/DVE and the
    # input DMAs finished, so keep only that queue's wait on the drain.
    import bass_rust

    out_q = None
    insts = []
    for bb in nc.m.functions[0].blocks:
        insts.extend(bb.instructions)
    for ins in insts:
        if type(ins).__name__ == "InstDMACopy" and ins.sync_info is not None:
            for u in ins.sync_info.on_update:
                if u.ant_name.startswith("DMAHW") or u.ant_name.startswith("DMASW"):
                    out_q = (u.ant_name, ins)
    assert out_q is not None
    qname, _ = out_q
    for ins in insts:
        if type(ins).__name__ == "InstDrain" and ins.sync_info is not None:
            kept = [w for w in ins.sync_info.on_wait if w.ant_name == qname]
            ins.sync_info = mybir.SyncInfo(on_wait=kept, on_update=list(ins.sync_info.on_update))
    return nc


def _fold_weights(w_b, w_s, control_points, g0, h, bound):
    """Host-side fold: 17 control points -> 9 GEMM weight blocks (float64 math).

    Features are computed on-device in t-units (tc = clip(x), u = tc - tctr,
    r_k = relu(tc - knot_k)); the 1/h^j scalings fold into the weights here.
    """
    D = w_s[:, :, None].astype(np.float64) * control_points.astype(np.float64)
    E = np.zeros((8, IN_DIM, OUT_DIM))
    for k in range(8):
        for c in range(max(0, k - 4), min(7, k) + 1):
            E[k] += D[:, :, c] * ((-1.0) ** (k - c) * comb(4, k - c) / 6.0)

    ctr = 5.0  # v-space center of the clipped data range [2.5, 7.5]
    # centered expansion of sum_{k=0,1,2} E_k (v-k)^3 in powers of (v - ctr)
    a = [ctr - 0.0, ctr - 1.0, ctr - 2.0]
    G3 = E[0] + E[1] + E[2]
    G2 = 3.0 * (a[0] * E[0] + a[1] * E[1] + a[2] * E[2])
    G1 = 3.0 * (a[0] ** 2 * E[0] + a[1] ** 2 * E[1] + a[2] ** 2 * E[2])
    G0 = a[0] ** 3 * E[0] + a[1] ** 3 * E[1] + a[2] ** 3 * E[2]
    Gsum0 = G0.sum(axis=0)

    W = np.zeros((NKT + 1, 128, OUT_DIM), np.float32)
    W[NKT] = 1.0  # ones-feature slice, rides the same DMA as the weights
    W[NKT - 1, 0, :] = Gsum0.astype(np.float32)
    blocks = [w_b.astype(np.float64), G1 / h, G2 / h**2, G3 / h**3] + [
        E[k] / h**3 for k in range(3, 8)
    ]
    for bi, blk in enumerate(blocks):
        W[bi * 4 : (bi + 1) * 4] = blk.reshape(4, 128, OUT_DIM).astype(np.float32)
    return W


last_results = None


def kernel(x, w_b, w_s, control_points, grid_points, bound):
    global last_results
    x = np.asarray(x, np.float32)
    w_b = np.asarray(w_b, np.float32)
    w_s = np.asarray(w_s, np.float32)
    control_points = np.asarray(control_points, np.float32)
    grid_points = np.asarray(grid_points, np.float64)
    bound = float(np.asarray(bound))

    g0 = float(grid_points[0])
    h = float((grid_points[-1] - grid_points[0]) / (len(grid_points) - 1))

    W = _fold_weights(w_b, w_s, control_points, g0, h, bound)
    if MM_DTYPE == "bf16":
        import ml_dtypes

        W = W.astype(ml_dtypes.bfloat16)

    key = (g0, h, bound, MM_DTYPE)
    if key not in _nc_cache:
        _nc_cache[key] = _build_nc(g0, h, bound)
    nc = _nc_cache[key]

    in_maps = []
    for k in range(NCORES):
        xt_k = np.ascontiguousarray(x[k * BC : (k + 1) * BC, :].T.reshape(4, 128, BC))
        in_maps.append({"xt": xt_k, "w": W})

    from concourse.bass_utils import run_bass_kernel_spmd

    last_results = run_bass_kernel_spmd(nc, in_maps, list(range(NCORES)))
    out = np.concatenate(
        [last_results.results[k]["out"].reshape(BC, OUT_DIM) for k in range(NCORES)],
        axis=0,
    )
    return out

